# revision 15
# baseline (speedup 1.0000x reference)
"""Trainium2 Bass kernel for nn_CustomMLPLayer (topk_masking).

Computes, for x [1, 2048, 11008] f32 and weight [4096, 11008] f32:
  true_value = einsum('bsn,mn->bsm', x, weight)            [1, 2048, 4096]
  core_idx   = top-4403 neurons by per-token top-2201 frequency
  filtered_W = weight[:, core_idx]                          [4096, 4403]

Sharding (8 cores, tensor-parallel per the d_model dim):
  - each core computes out[:, i*512:(i+1)*512] = x @ W_shard.T
  - each core also runs the per-token top-k threshold search for its
    256-token slice and emits partial neuron activation counts.

Device kernel per core:
  - matmul: out[2048, 512] = xt[11008, 2048]^T @ wt[11008, 512]
    (hand-rolled: 2 M-halves x 86 K-tiles x 8 PSUM banks)
  - bisection (22 iters) on the vector engine finds, per token, a
    threshold t with |{j : x[s,j] >= t}| == 2201 (the top-k boundary)
  - final pass: per-token count at t (n_ge) + per-neuron partial counts
    via a ones-vector matmul over the {0,1} masks.

Host: sums partial counts, repairs any token rows where n_ge != k
(top-k boundary ties / bracket misses), reproduces jax.lax.top_k
ordering with a stable argsort, and gathers filtered_W.
"""

import numpy as np

import concourse.bass as bass
import concourse.mybir as mybir
import concourse.tile as tile
from concourse import bacc
from concourse.bass_utils import run_bass_kernel_spmd

# ---- problem constants (hardcoded; kernel.py must be self-contained) ----
S = 2048            # tokens
N = 11008           # neurons (contraction dim)
M = 4096            # d_model
NC = 8              # cores
D = M // NC         # 512 d_model cols per core
ST = S // NC        # 256 tokens per core
KT = N // 128       # 86 k-tiles
MT = S // 128       # 16 token m-tiles (output partitions)
K_TOP = int(0.2 * N)       # 2201 per-token top-k
CORE_NUM = int(0.4 * N)    # 4403 core neurons
ITERS = 22
LO0 = 0.6
W0 = 0.25           # bracket [0.6, 1.1]

TRACE = False       # test harness may flip this for NTFF profiling
LAST_EXEC_TIME_NS = None

_F32 = mybir.dt.float32
_F32R = mybir.dt.float32r
_BF16 = mybir.dt.bfloat16


def _build_nc():
    nc = bacc.Bacc(trn_type="TRN2", target_bir_lowering=False, debug=False)

    xt = nc.dram_tensor("xt", [N, S], _F32R, kind="ExternalInput")
    wt = nc.dram_tensor("wt", [N, D], _F32R, kind="ExternalInput")
    xs = nc.dram_tensor("xs", [ST, N], _F32, kind="ExternalInput")

    out = nc.dram_tensor("out", [S, D], _F32, kind="ExternalOutput")
    pc = nc.dram_tensor("pc", [1, N], _F32, kind="ExternalOutput")
    nge = nc.dram_tensor("nge", [2, 128, 1], _F32, kind="ExternalOutput")
    tlo = nc.dram_tensor("tlo", [2, 128, 1], _F32, kind="ExternalOutput")

    with tile.TileContext(nc) as tc:
        with (
            tc.tile_pool(name="const", bufs=1) as const_pool,
            tc.tile_pool(name="xsp", bufs=1) as xs_pool,
            tc.tile_pool(name="bis", bufs=1) as bis_pool,
            tc.tile_pool(name="chk", bufs=3) as chk_pool,
            tc.tile_pool(name="bis_small", bufs=4) as small_pool,
            tc.tile_pool(name="wtp", bufs=3) as wt_pool,
            tc.tile_pool(name="xtp", bufs=3) as xt_pool,
            tc.tile_pool(name="evp", bufs=3) as ev_pool,
            tc.tile_pool(name="psum", bufs=8, space="PSUM") as psum_pool,
        ):
            # ---------------- bisection over per-token thresholds ----------
            # f32-only DVE ops run in 2x_2P mode; the throwaway mask is f32
            # and bitcast to f32r where it feeds the ones-matmul.
            ones_f = const_pool.tile([128, 1], _F32)
            nc.vector.memset(ones_f[:, :], 1.0)
            ones = const_pool.tile([128, 1], _F32R)
            nc.vector.tensor_copy(ones[:, :], ones_f[:, :])

            xs_tiles = []
            lo_tiles = []
            for t in range(2):
                xst = xs_pool.tile([128, N], _F32, name=f"xs{t}")
                nc.sync.dma_start(out=xst[:, :], in_=xs[t * 128 : (t + 1) * 128, :])
                xs_tiles.append(xst)
                lo = small_pool.tile([128, 1], _F32, name=f"lo{t}")
                nc.vector.memset(lo[:, :], LO0)
                lo_tiles.append(lo)

            # Column split across three engines, each counting its slice:
            # DVE / GPSIMD via is_ge+accum, ACT via Sign(x - t)+accum
            # (count_ge = (cols + sum_sign)/2 when no element equals t; any
            # x == t ambiguity is caught by the host's n_ge != k repair).
            DV = 4850
            AV = N - DV
            for it in range(ITERS):
                w = W0 * (2.0 ** (-it))
                for t in range(2):
                    lo = lo_tiles[t]
                    mid = small_pool.tile([128, 1], _F32, name=f"mid{t}")
                    nc.vector.tensor_scalar_add(mid[:, :], lo[:, :], w)
                    nmid = small_pool.tile([128, 1], _F32, name=f"nmid{t}")
                    nc.vector.tensor_scalar_mul(nmid[:, :], mid[:, :], -1.0)

                    mask_d = bis_pool.tile([128, DV], _F32, name="mask_d")
                    cd = small_pool.tile([128, 1], _F32, name=f"cd{t}")
                    nc.vector.tensor_scalar(
                        mask_d[:, :],
                        xs_tiles[t][:, 0:DV],
                        mid[:, :],
                        None,
                        mybir.AluOpType.is_ge,
                        mybir.AluOpType.add,
                        accum_out=cd[:, :],
                    )
                    mask_a = bis_pool.tile([128, AV], _F32, name="mask_a")
                    ca = small_pool.tile([128, 1], _F32, name=f"ca{t}")
                    nc.scalar.activation(
                        mask_a[:, :],
                        xs_tiles[t][:, DV:N],
                        mybir.ActivationFunctionType.Sign,
                        bias=nmid[:, :],
                        scale=1.0,
                        accum_out=ca[:, :],
                    )

                    cnt2 = small_pool.tile([128, 1], _F32, name=f"cnt2{t}")
                    nc.vector.scalar_tensor_tensor(
                        cnt2[:, :],
                        ca[:, :],
                        0.5,
                        cd[:, :],
                        mybir.AluOpType.mult,
                        mybir.AluOpType.add,
                    )
                    pred = small_pool.tile([128, 1], _F32, name=f"pred{t}")
                    nc.vector.tensor_scalar(
                        pred[:, :],
                        cnt2[:, :],
                        float(K_TOP) - AV / 2.0,
                        None,
                        mybir.AluOpType.is_ge,
                    )
                    newlo = small_pool.tile([128, 1], _F32, name=f"lo{t}")
                    nc.vector.scalar_tensor_tensor(
                        newlo[:, :],
                        pred[:, :],
                        w,
                        lo[:, :],
                        mybir.AluOpType.mult,
                        mybir.AluOpType.add,
                    )
                    lo_tiles[t] = newlo

            # ---------------- matmul out = xt^T @ wt ----------------------
            # Emitted before the counts phase so the matmul claims the PSUM
            # slots first; the counts-phase PSUM allocations then queue
            # behind the matmul evictions instead of the other way around.
            for half in range(2):
                psums = [
                    psum_pool.tile([128, D], _F32, name="ps") for m in range(8)
                ]
                for kt_i in range(KT):
                    k0 = kt_i * 128
                    if kt_i % 2 == 0:
                        wt_t = wt_pool.tile([128, 2, D], _F32R, name="wt_t")
                        nc.sync.dma_start(
                            out=wt_t[:, :, :],
                            in_=wt[k0 : k0 + 256, :].rearrange(
                                "(a p) d -> p a d", p=128
                            ),
                        )
                    xt_t = xt_pool.tile([128, 1024], _F32R, name="xt_t")
                    nc.sync.dma_start(
                        out=xt_t[:, :],
                        in_=xt[k0 : k0 + 128, half * 1024 : (half + 1) * 1024],
                    )
                    for m in range(8):
                        nc.tensor.matmul(
                            psums[m][:, :],
                            lhsT=xt_t[:, m * 128 : (m + 1) * 128],
                            rhs=wt_t[:, kt_i % 2, :],
                            start=(kt_i == 0),
                            stop=(kt_i == KT - 1),
                        )
                for m in range(8):
                    mg = half * 8 + m
                    ev = ev_pool.tile([128, D], _F32, name="ev")
                    nc.scalar.copy(ev[:, :], psums[m][:, :])
                    nc.sync.dma_start(
                        out=out[mg * 128 : (mg + 1) * 128, :], in_=ev[:, :]
                    )

            # ---------------- final pass: n_ge + partial counts ------------
            CH = 512
            n_chunks = (N + CH - 1) // CH
            nge_strips = []
            for t in range(2):
                strip = small_pool.tile([128, n_chunks], _F32, name=f"strip{t}")
                nge_strips.append(strip)

            for c in range(n_chunks):
                c0 = c * CH
                cw = min(CH, N - c0)
                cpsum = psum_pool.tile([128, CH], _F32, name="ps")
                for t in range(2):
                    mch = chk_pool.tile([128, CH], _F32R, name="mchunk")
                    nc.vector.tensor_scalar(
                        mch[:, :cw],
                        xs_tiles[t][:, c0 : c0 + cw],
                        lo_tiles[t][:, :],
                        None,
                        mybir.AluOpType.is_ge,
                        mybir.AluOpType.add,
                        accum_out=nge_strips[t][:, c : c + 1],
                    )
                    nc.tensor.matmul(
                        cpsum[0:1, :cw],
                        lhsT=ones[:, :],
                        rhs=mch[:, :cw],
                        start=(t == 0),
                        stop=(t == 1),
                    )
                cnt_ev = chk_pool.tile([1, CH], _F32, name="cnt_ev")
                nc.scalar.copy(cnt_ev[:, :cw], cpsum[0:1, :cw])
                nc.sync.dma_start(out=pc[:, c0 : c0 + cw], in_=cnt_ev[:, :cw])
            for t in range(2):
                nge_t = small_pool.tile([128, 1], _F32, name=f"nge{t}")
                nc.vector.tensor_reduce(
                    nge_t[:, :],
                    nge_strips[t][:, :],
                    mybir.AxisListType.X,
                    mybir.AluOpType.add,
                )
                nc.sync.dma_start(out=nge[t], in_=nge_t[:, :])
                nc.sync.dma_start(out=tlo[t], in_=lo_tiles[t][:, :])

    # Bacc's compile pass (register allocation + splitting sync waits to
    # one per instruction, a TRN2 hardware constraint) runs in finalize();
    # run_bass_via_pjrt does not call it itself.
    nc.finalize()
    return nc


_NC_CACHE = None


def _get_nc():
    global _NC_CACHE
    if _NC_CACHE is None:
        _NC_CACHE = _build_nc()
    return _NC_CACHE


def _exact_row_topk_mask(row):
    """Boolean mask of the top-K_TOP entries of `row`, ties resolved like
    jax.lax.top_k (higher value first; equal values -> lower index)."""
    idx = np.argsort(-row, kind="stable")[:K_TOP]
    mask = np.zeros(row.shape[0], dtype=np.int64)
    mask[idx] = 1
    return mask


def kernel(x, weight):
    global LAST_EXEC_TIME_NS
    x = np.asarray(x)
    weight = np.asarray(weight)
    x2d = np.ascontiguousarray(x.reshape(S, N))

    xt = np.ascontiguousarray(x2d.T)  # [N, S]

    in_maps = []
    for i in range(NC):
        wt_i = np.ascontiguousarray(weight[i * D : (i + 1) * D, :].T)  # [N, D]
        xs_i = x2d[i * ST : (i + 1) * ST, :]
        in_maps.append({"xt": xt, "wt": wt_i, "xs": xs_i})

    nc = _get_nc()
    res = run_bass_kernel_spmd(nc, in_maps, list(range(NC)), trace=TRACE)
    if TRACE:
        LAST_EXEC_TIME_NS = res.exec_time_ns
    results = res.results

    # true_value: concat d_model shards
    true_value = np.concatenate(
        [results[i]["out"] for i in range(NC)], axis=1
    ).reshape(1, S, M)

    # counts: sum per-core partials, then repair any token row whose
    # threshold count != K_TOP (top-k boundary tie or bracket miss).
    counts = np.zeros(N, dtype=np.int64)
    for i in range(NC):
        counts += np.rint(results[i]["pc"].reshape(N)).astype(np.int64)

    for i in range(NC):
        n_ge = np.rint(results[i]["nge"].reshape(ST)).astype(np.int64)
        t_lo = results[i]["tlo"].reshape(ST).astype(np.float32)
        bad = np.nonzero(n_ge != K_TOP)[0]
        for s_local in bad:
            row = x2d[i * ST + s_local, :]
            dev_mask = (row >= t_lo[s_local]).astype(np.int64)
            counts += _exact_row_topk_mask(row) - dev_mask

    # top-CORE_NUM neurons by count, jax.lax.top_k tie order
    core_idx = np.argsort(-counts, kind="stable")[:CORE_NUM]

    filtered_w = weight[:, core_idx]

    return true_value, filtered_w


# revision 16
# speedup vs baseline: 1.1291x; 1.1291x over previous
"""Trainium2 Bass kernel for nn_CustomMLPLayer (topk_masking).

Computes, for x [1, 2048, 11008] f32 and weight [4096, 11008] f32:
  true_value = einsum('bsn,mn->bsm', x, weight)            [1, 2048, 4096]
  core_idx   = top-4403 neurons by per-token top-2201 frequency
  filtered_W = weight[:, core_idx]                          [4096, 4403]

Sharding (8 cores, tensor-parallel per the d_model dim):
  - each core computes out[:, i*512:(i+1)*512] = x @ W_shard.T
  - each core also runs the per-token top-k threshold search for its
    256-token slice and emits partial neuron activation counts.

Device kernel per core:
  - matmul: out[2048, 512] = xt[11008, 2048]^T @ wt[11008, 512]
    (hand-rolled: 2 M-halves x 86 K-tiles x 8 PSUM banks)
  - bisection (22 iters) on the vector engine finds, per token, a
    threshold t with |{j : x[s,j] >= t}| == 2201 (the top-k boundary)
  - final pass: per-token count at t (n_ge) + per-neuron partial counts
    via a ones-vector matmul over the {0,1} masks.

Host: sums partial counts, repairs any token rows where n_ge != k
(top-k boundary ties / bracket misses), reproduces jax.lax.top_k
ordering with a stable argsort, and gathers filtered_W.
"""

import numpy as np

import concourse.bass as bass
import concourse.mybir as mybir
import concourse.tile as tile
from concourse import bacc
from concourse.bass_utils import run_bass_kernel_spmd

# ---- problem constants (hardcoded; kernel.py must be self-contained) ----
S = 2048            # tokens
N = 11008           # neurons (contraction dim)
M = 4096            # d_model
NC = 8              # cores
D = M // NC         # 512 d_model cols per core
ST = S // NC        # 256 tokens per core
KT = N // 128       # 86 k-tiles
MT = S // 128       # 16 token m-tiles (output partitions)
K_TOP = int(0.2 * N)       # 2201 per-token top-k
CORE_NUM = int(0.4 * N)    # 4403 core neurons
ITERS = 22
LO0 = 0.6
W0 = 0.25           # bracket [0.6, 1.1]

TRACE = False       # test harness may flip this for NTFF profiling
LAST_EXEC_TIME_NS = None

_F32 = mybir.dt.float32
_F32R = mybir.dt.float32r
_BF16 = mybir.dt.bfloat16


def _build_nc():
    nc = bacc.Bacc(trn_type="TRN2", target_bir_lowering=False, debug=False)

    xt = nc.dram_tensor("xt", [2, 128, KT, 1024], _F32R, kind="ExternalInput")
    wt = nc.dram_tensor("wt", [128, KT, D], _F32R, kind="ExternalInput")
    xs = nc.dram_tensor("xs", [ST, N], _F32, kind="ExternalInput")

    out = nc.dram_tensor("out", [S, D], _F32, kind="ExternalOutput")
    pc = nc.dram_tensor("pc", [1, N], _F32, kind="ExternalOutput")
    nge = nc.dram_tensor("nge", [2, 128, 1], _F32, kind="ExternalOutput")
    tlo = nc.dram_tensor("tlo", [2, 128, 1], _F32, kind="ExternalOutput")

    with tile.TileContext(nc) as tc:
        with (
            tc.tile_pool(name="const", bufs=1) as const_pool,
            tc.tile_pool(name="xsp", bufs=1) as xs_pool,
            tc.tile_pool(name="bis", bufs=1) as bis_pool,
            tc.tile_pool(name="chk", bufs=2) as chk_pool,
            tc.tile_pool(name="bis_small", bufs=4) as small_pool,
            tc.tile_pool(name="wtp", bufs=2) as wt_pool,
            tc.tile_pool(name="xtp", bufs=2) as xt_pool,
            tc.tile_pool(name="evp", bufs=2) as ev_pool,
            tc.tile_pool(name="psum", bufs=8, space="PSUM") as psum_pool,
        ):
            # ---------------- bisection over per-token thresholds ----------
            # f32-only DVE ops run in 2x_2P mode; the throwaway mask is f32
            # and bitcast to f32r where it feeds the ones-matmul.
            ones_f = const_pool.tile([128, 1], _F32)
            nc.vector.memset(ones_f[:, :], 1.0)
            ones = const_pool.tile([128, 1], _F32R)
            nc.vector.tensor_copy(ones[:, :], ones_f[:, :])

            xs_tiles = []
            lo_tiles = []
            for t in range(2):
                xst = xs_pool.tile([128, N], _F32, name=f"xs{t}")
                nc.sync.dma_start(out=xst[:, :], in_=xs[t * 128 : (t + 1) * 128, :])
                xs_tiles.append(xst)
                lo = small_pool.tile([128, 1], _F32, name=f"lo{t}")
                nc.vector.memset(lo[:, :], LO0)
                lo_tiles.append(lo)

            # Column split across three engines, each counting its slice:
            # DVE / GPSIMD via is_ge+accum, ACT via Sign(x - t)+accum
            # (count_ge = (cols + sum_sign)/2 when no element equals t; any
            # x == t ambiguity is caught by the host's n_ge != k repair).
            DV = 3200
            AV = N - DV
            for it in range(ITERS):
                w = W0 * (2.0 ** (-it))
                for t in range(2):
                    lo = lo_tiles[t]
                    mid = small_pool.tile([128, 1], _F32, name=f"mid{t}")
                    nc.vector.tensor_scalar_add(mid[:, :], lo[:, :], w)
                    nmid = small_pool.tile([128, 1], _F32, name=f"nmid{t}")
                    nc.vector.tensor_scalar_mul(nmid[:, :], mid[:, :], -1.0)

                    mask_d = bis_pool.tile([128, DV], _F32, name="mask_d")
                    cd = small_pool.tile([128, 1], _F32, name=f"cd{t}")
                    nc.vector.tensor_scalar(
                        mask_d[:, :],
                        xs_tiles[t][:, 0:DV],
                        mid[:, :],
                        None,
                        mybir.AluOpType.is_ge,
                        mybir.AluOpType.add,
                        accum_out=cd[:, :],
                    )
                    mask_a = bis_pool.tile([128, AV], _BF16, name="mask_a")
                    ca = small_pool.tile([128, 1], _F32, name=f"ca{t}")
                    nc.scalar.activation(
                        mask_a[:, :],
                        xs_tiles[t][:, DV:N],
                        mybir.ActivationFunctionType.Sign,
                        bias=nmid[:, :],
                        scale=1.0,
                        accum_out=ca[:, :],
                    )

                    cnt2 = small_pool.tile([128, 1], _F32, name=f"cnt2{t}")
                    nc.vector.scalar_tensor_tensor(
                        cnt2[:, :],
                        ca[:, :],
                        0.5,
                        cd[:, :],
                        mybir.AluOpType.mult,
                        mybir.AluOpType.add,
                    )
                    pred = small_pool.tile([128, 1], _F32, name=f"pred{t}")
                    nc.vector.tensor_scalar(
                        pred[:, :],
                        cnt2[:, :],
                        float(K_TOP) - AV / 2.0,
                        None,
                        mybir.AluOpType.is_ge,
                    )
                    newlo = small_pool.tile([128, 1], _F32, name=f"lo{t}")
                    nc.vector.scalar_tensor_tensor(
                        newlo[:, :],
                        pred[:, :],
                        w,
                        lo[:, :],
                        mybir.AluOpType.mult,
                        mybir.AluOpType.add,
                    )
                    lo_tiles[t] = newlo

            # ---------------- matmul out = xt^T @ wt ----------------------
            # Emitted before the counts phase so the matmul claims the PSUM
            # slots first; the counts-phase PSUM allocations then queue
            # behind the matmul evictions instead of the other way around.
            # xt/wt arrive host-packed as [2, 128, KT, 1024] / [128, KT, D]
            # so each DMA row is one long contiguous run (16KB / 8KB).
            kt_groups = []
            kt0 = 0
            while kt0 < KT:
                g = min(4, KT - kt0)
                kt_groups.append((kt0, g))
                kt0 += g
            for half in range(2):
                psums = [
                    psum_pool.tile([128, D], _F32, name="ps") for m in range(8)
                ]
                for g0, gn in kt_groups:
                    wt_t = wt_pool.tile([128, 4, D], _F32R, name="wt_t")
                    nc.sync.dma_start(
                        out=wt_t[:, :gn, :], in_=wt[:, g0 : g0 + gn, :]
                    )
                    xt_t = xt_pool.tile([128, 4, 1024], _F32R, name="xt_t")
                    nc.sync.dma_start(
                        out=xt_t[:, :gn, :], in_=xt[half, :, g0 : g0 + gn, :]
                    )
                    for kk in range(gn):
                        kt_i = g0 + kk
                        for m in range(8):
                            nc.tensor.matmul(
                                psums[m][:, :],
                                lhsT=xt_t[:, kk, m * 128 : (m + 1) * 128],
                                rhs=wt_t[:, kk, :],
                                start=(kt_i == 0),
                                stop=(kt_i == KT - 1),
                            )
                for m in range(8):
                    mg = half * 8 + m
                    ev = ev_pool.tile([128, D], _F32, name="ev")
                    nc.scalar.copy(ev[:, :], psums[m][:, :])
                    nc.sync.dma_start(
                        out=out[mg * 128 : (mg + 1) * 128, :], in_=ev[:, :]
                    )

            # ---------------- final pass: n_ge + partial counts ------------
            CH = 512
            n_chunks = (N + CH - 1) // CH
            nge_strips = []
            for t in range(2):
                strip = small_pool.tile([128, n_chunks], _F32, name=f"strip{t}")
                nge_strips.append(strip)

            for c in range(n_chunks):
                c0 = c * CH
                cw = min(CH, N - c0)
                cpsum = psum_pool.tile([128, CH], _F32, name="ps")
                for t in range(2):
                    mch = chk_pool.tile([128, CH], _F32R, name="mchunk")
                    nc.vector.tensor_scalar(
                        mch[:, :cw],
                        xs_tiles[t][:, c0 : c0 + cw],
                        lo_tiles[t][:, :],
                        None,
                        mybir.AluOpType.is_ge,
                        mybir.AluOpType.add,
                        accum_out=nge_strips[t][:, c : c + 1],
                    )
                    nc.tensor.matmul(
                        cpsum[0:1, :cw],
                        lhsT=ones[:, :],
                        rhs=mch[:, :cw],
                        start=(t == 0),
                        stop=(t == 1),
                    )
                cnt_ev = chk_pool.tile([1, CH], _F32, name="cnt_ev")
                nc.scalar.copy(cnt_ev[:, :cw], cpsum[0:1, :cw])
                nc.sync.dma_start(out=pc[:, c0 : c0 + cw], in_=cnt_ev[:, :cw])
            for t in range(2):
                nge_t = small_pool.tile([128, 1], _F32, name=f"nge{t}")
                nc.vector.tensor_reduce(
                    nge_t[:, :],
                    nge_strips[t][:, :],
                    mybir.AxisListType.X,
                    mybir.AluOpType.add,
                )
                nc.sync.dma_start(out=nge[t], in_=nge_t[:, :])
                nc.sync.dma_start(out=tlo[t], in_=lo_tiles[t][:, :])

    # Bacc's compile pass (register allocation + splitting sync waits to
    # one per instruction, a TRN2 hardware constraint) runs in finalize();
    # run_bass_via_pjrt does not call it itself.
    nc.finalize()
    return nc


_NC_CACHE = None


def _get_nc():
    global _NC_CACHE
    if _NC_CACHE is None:
        _NC_CACHE = _build_nc()
    return _NC_CACHE


def _exact_row_topk_mask(row):
    """Boolean mask of the top-K_TOP entries of `row`, ties resolved like
    jax.lax.top_k (higher value first; equal values -> lower index)."""
    idx = np.argsort(-row, kind="stable")[:K_TOP]
    mask = np.zeros(row.shape[0], dtype=np.int64)
    mask[idx] = 1
    return mask


def kernel(x, weight):
    global LAST_EXEC_TIME_NS
    x = np.asarray(x)
    weight = np.asarray(weight)
    x2d = np.ascontiguousarray(x.reshape(S, N))

    # packed xt: xt_p[h, p, kt, j] = x2d[h*1024 + j, kt*128 + p] so each
    # (partition, dma) run in DRAM is one long contiguous stretch.
    xt_p = np.ascontiguousarray(
        x2d.reshape(2, 1024, KT, 128).transpose(0, 3, 2, 1)
    )  # [2, 128, KT, 1024]

    in_maps = []
    for i in range(NC):
        w_i = weight[i * D : (i + 1) * D, :]  # [D, N]
        # wt_p[p, kt, d] = w_i[d, kt*128 + p]
        wt_i = np.ascontiguousarray(w_i.reshape(D, KT, 128).transpose(2, 1, 0))
        xs_i = x2d[i * ST : (i + 1) * ST, :]
        in_maps.append({"xt": xt_p, "wt": wt_i, "xs": xs_i})

    nc = _get_nc()
    res = run_bass_kernel_spmd(nc, in_maps, list(range(NC)), trace=TRACE)
    if TRACE:
        LAST_EXEC_TIME_NS = res.exec_time_ns
    results = res.results

    # true_value: concat d_model shards
    true_value = np.concatenate(
        [results[i]["out"] for i in range(NC)], axis=1
    ).reshape(1, S, M)

    # counts: sum per-core partials, then repair any token row whose
    # threshold count != K_TOP (top-k boundary tie or bracket miss).
    counts = np.zeros(N, dtype=np.int64)
    for i in range(NC):
        counts += np.rint(results[i]["pc"].reshape(N)).astype(np.int64)

    for i in range(NC):
        n_ge = np.rint(results[i]["nge"].reshape(ST)).astype(np.int64)
        t_lo = results[i]["tlo"].reshape(ST).astype(np.float32)
        bad = np.nonzero(n_ge != K_TOP)[0]
        for s_local in bad:
            row = x2d[i * ST + s_local, :]
            dev_mask = (row >= t_lo[s_local]).astype(np.int64)
            counts += _exact_row_topk_mask(row) - dev_mask

    # top-CORE_NUM neurons by count, jax.lax.top_k tie order
    core_idx = np.argsort(-counts, kind="stable")[:CORE_NUM]

    filtered_w = weight[:, core_idx]

    return true_value, filtered_w


# revision 17
# speedup vs baseline: 1.2601x; 1.1160x over previous
"""Trainium2 Bass kernel for nn_CustomMLPLayer (topk_masking).

Computes, for x [1, 2048, 11008] f32 and weight [4096, 11008] f32:
  true_value = einsum('bsn,mn->bsm', x, weight)            [1, 2048, 4096]
  core_idx   = top-4403 neurons by per-token top-2201 frequency
  filtered_W = weight[:, core_idx]                          [4096, 4403]

Sharding (8 cores, tensor-parallel per the d_model dim):
  - each core computes out[:, i*512:(i+1)*512] = x @ W_shard.T
  - each core also runs the per-token top-k threshold search for its
    256-token slice and emits partial neuron activation counts.

Device kernel per core:
  - matmul: out[2048, 512] = xt[11008, 2048]^T @ wt[11008, 512]
    (hand-rolled: 2 M-halves x 86 K-tiles x 8 PSUM banks)
  - bisection (22 iters) on the vector engine finds, per token, a
    threshold t with |{j : x[s,j] >= t}| == 2201 (the top-k boundary)
  - final pass: per-token count at t (n_ge) + per-neuron partial counts
    via a ones-vector matmul over the {0,1} masks.

Host: sums partial counts, repairs any token rows where n_ge != k
(top-k boundary ties / bracket misses), reproduces jax.lax.top_k
ordering with a stable argsort, and gathers filtered_W.
"""

import numpy as np

import concourse.bass as bass
import concourse.mybir as mybir
import concourse.tile as tile
import concourse.bass_isa as bass_isa
from concourse import bacc
from concourse.bass_utils import run_bass_kernel_spmd

# ---- problem constants (hardcoded; kernel.py must be self-contained) ----
S = 2048            # tokens
N = 11008           # neurons (contraction dim)
M = 4096            # d_model
NC = 8              # cores
D = M // NC         # 512 d_model cols per core
ST = S // NC        # 256 tokens per core
KT = N // 128       # 86 k-tiles
MT = S // 128       # 16 token m-tiles (output partitions)
K_TOP = int(0.2 * N)       # 2201 per-token top-k
CORE_NUM = int(0.4 * N)    # 4403 core neurons
ITERS = 20
LO0 = 0.6
W0 = 0.25           # bracket [0.6, 1.1]

TRACE = False       # test harness may flip this for NTFF profiling
LAST_EXEC_TIME_NS = None

_F32 = mybir.dt.float32
_F32R = mybir.dt.float32r
_BF16 = mybir.dt.bfloat16


def _build_nc():
    nc = bacc.Bacc(trn_type="TRN2", target_bir_lowering=False, debug=False)

    xt = nc.dram_tensor("xt", [2, 128, KT, 1024], _F32R, kind="ExternalInput")
    wt = nc.dram_tensor("wt", [128, KT, D], _F32R, kind="ExternalInput")
    xs = nc.dram_tensor("xs", [ST, N], _F32, kind="ExternalInput")

    out = nc.dram_tensor("out", [S, D], _F32, kind="ExternalOutput")
    pc = nc.dram_tensor("pc", [1, N], _F32, kind="ExternalOutput")
    nge = nc.dram_tensor("nge", [2, 128, 1], _F32, kind="ExternalOutput")
    tlo = nc.dram_tensor("tlo", [2, 128, 1], _F32, kind="ExternalOutput")

    with tile.TileContext(nc) as tc:
        with (
            tc.tile_pool(name="const", bufs=1) as const_pool,
            tc.tile_pool(name="xsp", bufs=1) as xs_pool,
            tc.tile_pool(name="bis", bufs=1) as bis_pool,
            tc.tile_pool(name="chk", bufs=2) as chk_pool,
            tc.tile_pool(name="bis_small", bufs=4) as small_pool,
            tc.tile_pool(name="wtp", bufs=2) as wt_pool,
            tc.tile_pool(name="xtp", bufs=2) as xt_pool,
            tc.tile_pool(name="evp", bufs=2) as ev_pool,
            tc.tile_pool(name="psum", bufs=8, space="PSUM") as psum_pool,
        ):
            # ---------------- bisection over per-token thresholds ----------
            # f32-only DVE ops run in 2x_2P mode; the throwaway mask is f32
            # and bitcast to f32r where it feeds the ones-matmul.
            xs_tiles = []
            lo_tiles = []
            for t in range(2):
                xst = xs_pool.tile([128, N], _F32, name=f"xs{t}")
                nc.sync.dma_start(out=xst[:, :], in_=xs[t * 128 : (t + 1) * 128, :])
                xs_tiles.append(xst)
                lo = small_pool.tile([128, 1], _F32, name=f"lo{t}")
                nc.vector.memset(lo[:, :], LO0)
                lo_tiles.append(lo)

            # Column split across three engines, each counting its slice:
            # DVE / GPSIMD via is_ge+accum, ACT via Sign(x - t)+accum
            # (count_ge = (cols + sum_sign)/2 when no element equals t; any
            # x == t ambiguity is caught by the host's n_ge != k repair).
            DV = 4600
            AV = N - DV
            for it in range(ITERS):
                w = W0 * (2.0 ** (-it))
                for t in range(2):
                    lo = lo_tiles[t]
                    mid = small_pool.tile([128, 1], _F32, name=f"mid{t}")
                    nc.vector.tensor_scalar_add(mid[:, :], lo[:, :], w)
                    mask_d = bis_pool.tile([128, DV], _F32, name="mask_d")
                    cd = small_pool.tile([128, 1], _F32, name=f"cd{t}")
                    nc.vector.tensor_scalar(
                        mask_d[:, :],
                        xs_tiles[t][:, 0:DV],
                        mid[:, :],
                        None,
                        mybir.AluOpType.is_ge,
                        mybir.AluOpType.add,
                        accum_out=cd[:, :],
                    )
                    mask_a = bis_pool.tile([128, AV], _BF16, name="mask_a")
                    ca = small_pool.tile([128, 1], _F32, name=f"ca{t}")
                    nc.scalar.activation(
                        mask_a[:, :],
                        xs_tiles[t][:, DV:N],
                        mybir.ActivationFunctionType.Sign,
                        bias=mid[:, :],
                        scale=-1.0,
                        accum_out=ca[:, :],
                    )

                    cnt2 = small_pool.tile([128, 1], _F32, name=f"cnt2{t}")
                    nc.vector.scalar_tensor_tensor(
                        cnt2[:, :],
                        ca[:, :],
                        -0.5,
                        cd[:, :],
                        mybir.AluOpType.mult,
                        mybir.AluOpType.add,
                    )
                    pred = small_pool.tile([128, 1], _F32, name=f"pred{t}")
                    nc.vector.tensor_scalar(
                        pred[:, :],
                        cnt2[:, :],
                        float(K_TOP) - AV / 2.0,
                        None,
                        mybir.AluOpType.is_ge,
                    )
                    newlo = small_pool.tile([128, 1], _F32, name=f"lo{t}")
                    nc.vector.scalar_tensor_tensor(
                        newlo[:, :],
                        pred[:, :],
                        w,
                        lo[:, :],
                        mybir.AluOpType.mult,
                        mybir.AluOpType.add,
                    )
                    lo_tiles[t] = newlo

            # ---------------- matmul out = xt^T @ wt ----------------------
            # Emitted before the counts phase so the matmul claims the PSUM
            # slots first; the counts-phase PSUM allocations then queue
            # behind the matmul evictions instead of the other way around.
            # xt/wt arrive host-packed as [2, 128, KT, 1024] / [128, KT, D]
            # so each DMA row is one long contiguous run (16KB / 8KB).
            kt_groups = []
            kt0 = 0
            while kt0 < KT:
                g = min(4, KT - kt0)
                kt_groups.append((kt0, g))
                kt0 += g
            for half in range(2):
                psums = [
                    psum_pool.tile([128, D], _F32, name="ps") for m in range(8)
                ]
                for g0, gn in kt_groups:
                    wt_t = wt_pool.tile([128, 4, D], _F32R, name="wt_t")
                    nc.sync.dma_start(
                        out=wt_t[:, :gn, :], in_=wt[:, g0 : g0 + gn, :]
                    )
                    xt_t = xt_pool.tile([128, 4, 1024], _F32R, name="xt_t")
                    nc.sync.dma_start(
                        out=xt_t[:, :gn, :], in_=xt[half, :, g0 : g0 + gn, :]
                    )
                    for kk in range(gn):
                        kt_i = g0 + kk
                        for m in range(8):
                            nc.tensor.matmul(
                                psums[m][:, :],
                                lhsT=xt_t[:, kk, m * 128 : (m + 1) * 128],
                                rhs=wt_t[:, kk, :],
                                start=(kt_i == 0),
                                stop=(kt_i == KT - 1),
                            )
                for m in range(8):
                    mg = half * 8 + m
                    ev = ev_pool.tile([128, D], _F32, name="ev")
                    nc.scalar.copy(ev[:, :], psums[m][:, :])
                    nc.sync.dma_start(
                        out=out[mg * 128 : (mg + 1) * 128, :], in_=ev[:, :]
                    )

            # ---------------- final pass: n_ge + partial counts ------------
            CH = 512
            n_chunks = (N + CH - 1) // CH
            nge_strips = []
            for t in range(2):
                strip = small_pool.tile([128, n_chunks], _F32, name=f"strip{t}")
                nge_strips.append(strip)

            for c in range(n_chunks):
                c0 = c * CH
                cw = min(CH, N - c0)
                mchs = []
                for t in range(2):
                    mch = chk_pool.tile([128, CH], _F32, name=f"mchunk{t}")
                    nc.vector.tensor_scalar(
                        mch[:, :cw],
                        xs_tiles[t][:, c0 : c0 + cw],
                        lo_tiles[t][:, :],
                        None,
                        mybir.AluOpType.is_ge,
                        mybir.AluOpType.add,
                        accum_out=nge_strips[t][:, c : c + 1],
                    )
                    mchs.append(mch)
                msum = chk_pool.tile([128, CH], _F32, name="msum")
                nc.vector.tensor_tensor(
                    msum[:, :cw], mchs[0][:, :cw], mchs[1][:, :cw],
                    mybir.AluOpType.add,
                )
                pcred = chk_pool.tile([128, CH], _F32, name="pcred")
                nc.gpsimd.partition_all_reduce(
                    pcred[:, :cw], msum[:, :cw], 128, bass_isa.ReduceOp.add
                )
                nc.sync.dma_start(out=pc[:, c0 : c0 + cw], in_=pcred[0:1, :cw])
            for t in range(2):
                nge_t = small_pool.tile([128, 1], _F32, name=f"nge{t}")
                nc.vector.tensor_reduce(
                    nge_t[:, :],
                    nge_strips[t][:, :],
                    mybir.AxisListType.X,
                    mybir.AluOpType.add,
                )
                nc.sync.dma_start(out=nge[t], in_=nge_t[:, :])
                nc.sync.dma_start(out=tlo[t], in_=lo_tiles[t][:, :])

    # Bacc's compile pass (register allocation + splitting sync waits to
    # one per instruction, a TRN2 hardware constraint) runs in finalize();
    # run_bass_via_pjrt does not call it itself.
    nc.finalize()
    return nc


_NC_CACHE = None


def _get_nc():
    global _NC_CACHE
    if _NC_CACHE is None:
        _NC_CACHE = _build_nc()
    return _NC_CACHE


def _exact_row_topk_mask(row):
    """Boolean mask of the top-K_TOP entries of `row`, ties resolved like
    jax.lax.top_k (higher value first; equal values -> lower index)."""
    idx = np.argsort(-row, kind="stable")[:K_TOP]
    mask = np.zeros(row.shape[0], dtype=np.int64)
    mask[idx] = 1
    return mask


def kernel(x, weight):
    global LAST_EXEC_TIME_NS
    x = np.asarray(x)
    weight = np.asarray(weight)
    x2d = np.ascontiguousarray(x.reshape(S, N))

    # packed xt: xt_p[h, p, kt, j] = x2d[h*1024 + j, kt*128 + p] so each
    # (partition, dma) run in DRAM is one long contiguous stretch.
    xt_p = np.ascontiguousarray(
        x2d.reshape(2, 1024, KT, 128).transpose(0, 3, 2, 1)
    )  # [2, 128, KT, 1024]

    in_maps = []
    for i in range(NC):
        w_i = weight[i * D : (i + 1) * D, :]  # [D, N]
        # wt_p[p, kt, d] = w_i[d, kt*128 + p]
        wt_i = np.ascontiguousarray(w_i.reshape(D, KT, 128).transpose(2, 1, 0))
        xs_i = x2d[i * ST : (i + 1) * ST, :]
        in_maps.append({"xt": xt_p, "wt": wt_i, "xs": xs_i})

    nc = _get_nc()
    res = run_bass_kernel_spmd(nc, in_maps, list(range(NC)), trace=TRACE)
    if TRACE:
        LAST_EXEC_TIME_NS = res.exec_time_ns
    results = res.results

    # true_value: concat d_model shards
    true_value = np.concatenate(
        [results[i]["out"] for i in range(NC)], axis=1
    ).reshape(1, S, M)

    # counts: sum per-core partials, then repair any token row whose
    # threshold count != K_TOP (top-k boundary tie or bracket miss).
    counts = np.zeros(N, dtype=np.int64)
    for i in range(NC):
        counts += np.rint(results[i]["pc"].reshape(N)).astype(np.int64)

    for i in range(NC):
        n_ge = np.rint(results[i]["nge"].reshape(ST)).astype(np.int64)
        t_lo = results[i]["tlo"].reshape(ST).astype(np.float32)
        bad = np.nonzero(n_ge != K_TOP)[0]
        for s_local in bad:
            row = x2d[i * ST + s_local, :]
            dev_mask = (row >= t_lo[s_local]).astype(np.int64)
            counts += _exact_row_topk_mask(row) - dev_mask

    # top-CORE_NUM neurons by count, jax.lax.top_k tie order
    core_idx = np.argsort(-counts, kind="stable")[:CORE_NUM]

    filtered_w = weight[:, core_idx]

    return true_value, filtered_w
